# revision 7
# baseline (speedup 1.0000x reference)
"""CARAFE content-aware upsampling on 8 Trainium2 NeuronCores (Bass/Tile).

Problem: features (4,128,64,64) f32, masks (4,25,128,128) f32
         -> out (4,128,128,128) f32
out[n,c,2h+a,2w+b] = sum_{i,j in 5x5} f[n,c,h+i-2,w+j-2] * m[n,5i+j,2h+a,2w+b]

v2 strategy (per core = one (n, h-half) shard), all-bf16 on device:
  Weight-stationary over shard feature rows r (36 of them): one bf16
  LDWEIGHTS of fT_row(r) [w''(68), c(128)] feeds up to 5 accumulating
  matmuls, one per in-flight job h = r-4..r (i = r-h), each into its own
  PSUM tile:  out_job(h) [c, (a,wup)=256] += fT_row(h+i).T @ B_i(h).
  B_i is the banded mask matrix [68, 256-in-288-region]; bands for 4 jobs
  are materialized per batch by ONE 4-dim SBUF->SBUF diagonal-scatter DMA
  on the HWDGE rings (sync/scalar), descriptors = 68x4x5 runs of 20 bf16.
  Band background zeros are memset once (static sparsity; runs overwrite
  in place), split across vector/gpsimd so the tensor engine starts ~2us
  in.  PSUM f32 results are cast-copied to bf16 staging on vector and
  DMA'd out 8 jobs (16 upsampled rows) per store on gpsimd; the host
  upcasts to f32.  bf16 keeps matmul at 1 col/cycle, halves LDWEIGHTS
  streaming and all DMA bytes; rel err ~3e-3 vs the 2e-2 gate.
"""
import sys

if "/opt/trn_rl_repo" not in sys.path:
    sys.path.insert(0, "/opt/trn_rl_repo")

from contextlib import ExitStack

import ml_dtypes
import numpy as np

import concourse.tile as tile
from concourse import bacc, mybir
from concourse.ap import AP
from concourse.bass_utils import run_bass_kernel_spmd

# ---- problem constants (hardcoded per harness contract) ----
N = 4
C = 128
H = 64
W = 64
KS = 5
PAD = 2
SCALE = 2
WP = W + KS - 1          # 68 contraction width per feature row
NB = SCALE * W           # 128 upsampled cols per hup row
RUN = 4 * KS             # 20 elems per diagonal run (w,b,a interleaved)
REG = 2 * NB + 32        # 288 per-band region: 16 pad | 256 data | 16 pad
BW = KS * REG            # 1440 band elems per job
NH = H // 2              # 32 low-res rows (jobs) per core
NROWS = NH + 4           # 36 feature rows per shard (halo zero-padded)
JPB = 4                  # jobs per scatter batch == band bufs per group
NGRP = 3                 # band tile groups (12 bufs total)
GW = JPB * BW            # free width of one band group tile
MSK_COLS = NH * KS * RUN

F32 = mybir.dt.float32
BF16 = mybir.dt.bfloat16

_PROG_CACHE: dict = {}


def _device_body(tc, ctx, out_ap, ft_ap, msk3_ap):
    nc = tc.nc
    sb = ctx.enter_context(tc.tile_pool(name="sb", bufs=1))
    psum = ctx.enter_context(tc.tile_pool(name="ps", bufs=8, space="PSUM"))
    obp = ctx.enter_context(tc.tile_pool(name="ob", bufs=2))

    ft = sb.tile([WP, NROWS * C], BF16)
    mst = sb.tile([WP, MSK_COLS], BF16)
    bands = [
        sb.tile([WP, GW], BF16, name=f"bg{g}", tag=f"bg{g}") for g in range(NGRP)
    ]

    # chunked input loads on the two HWDGE rings; small first chunks unblock
    # job 0 as early as possible
    nc.sync.dma_start(mst[:, : 8 * KS * RUN], msk3_ap[:, : 8 * KS * RUN])
    nc.scalar.dma_start(ft[:, : 6 * C], ft_ap[:, : 6 * C])
    nc.sync.dma_start(mst[:, 8 * KS * RUN :], msk3_ap[:, 8 * KS * RUN :])
    nc.scalar.dma_start(ft[:, 6 * C : 18 * C], ft_ap[:, 6 * C : 18 * C])
    nc.scalar.dma_start(ft[:, 18 * C :], ft_ap[:, 18 * C :])

    # band zero-fill split across three engines (static sparsity: done once)
    nc.vector.memset(bands[0][:, : 2 * BW], 0.0)
    nc.gpsimd.memset(bands[0][:, 2 * BW :], 0.0)
    nc.vector.memset(bands[1][:, : 2 * BW], 0.0)
    nc.gpsimd.memset(bands[1][:, 2 * BW :], 0.0)
    nc.scalar.memzero(bands[2][:, : 2 * BW])
    nc.scalar.memzero(bands[2][:, 2 * BW :])

    def scatter(b):
        # batch b: jobs 4b..4b+3 -> group b%NGRP, one 4-dim diagonal DMA
        g = bands[b % NGRP][:]
        dst = AP(g.tensor, g.offset, [[GW + 4, WP], [BW, JPB], [REG, KS], [1, RUN]])
        m = mst[:]
        src = AP(
            m.tensor,
            m.offset + 4 * b * KS * RUN,
            [[MSK_COLS, WP], [KS * RUN, JPB], [RUN, KS], [1, RUN]],
        )
        eng = nc.sync if b % 2 == 0 else nc.scalar
        eng.dma_start(dst, src)

    scatter(0)
    scatter(1)
    scatter(2)

    pt = {}
    ob = None
    for r in range(NROWS):
        lhsT = ft[:, r * C : (r + 1) * C]
        for h in range(max(0, r - 4), min(NH - 1, r) + 1):
            i = r - h
            if i == 0:
                pt[h] = psum.tile([C, 2 * NB], F32, name=f"pt{h}", tag="pt")
            g = bands[(h // JPB) % NGRP][:]
            # iterate the 256 band cols in ascending (contiguous) order:
            # psum col = 2*wup + a, de-interleaved by the output copy
            rhs = AP(
                g.tensor,
                g.offset + (h % JPB) * BW + i * REG + 16,
                [[GW, WP], [2, NB], [1, 2]],
            )
            nc.tensor.matmul(pt[h][:], lhsT, rhs, start=(i == 0), stop=(i == 4))

        # band batch b is needed from r=4b; its group was last read at
        # r=4b-5 (jobs 4b-12..4b-9), so emit right after that row.
        b = (r + 5) // 4
        if (r + 5) % 4 == 0 and 3 <= b < 8:
            scatter(b)

        if r >= 4:
            j = r - 4  # job whose accumulation just finished
            if j % 8 == 0:
                ob = obp.tile([C, 8 * 2 * NB], BF16, name=f"ob{j // 8}", tag="ob")
            p = pt[j][:]
            src = AP(p.tensor, p.offset, [[2 * NB, C], [1, 2], [2, NB]])
            nc.vector.tensor_copy(
                ob[:, (j % 8) * 2 * NB : (j % 8 + 1) * 2 * NB], src
            )
            del pt[j]
            if j % 8 == 7:
                q = j // 8
                nc.gpsimd.dma_start(out_ap[:, 16 * q : 16 * q + 16, :], ob[:])


def _build_program():
    nc = bacc.Bacc(
        "TRN2", debug=False, enable_asserts=False, target_bir_lowering=False
    )
    ft_t = nc.dram_tensor("ft", [WP, NROWS * C], BF16, kind="ExternalInput")
    msk_t = nc.dram_tensor("msk3", [WP, MSK_COLS], BF16, kind="ExternalInput")
    out_t = nc.dram_tensor("out", [C, 2 * NH, NB], BF16, kind="ExternalOutput")

    with tile.TileContext(nc) as tc, ExitStack() as ctx:
        _device_body(tc, ctx, out_t.ap(), ft_t.ap(), msk_t.ap())
    nc.compile()
    return nc


def _prep_ft(feat_n: np.ndarray, h0: int) -> np.ndarray:
    """[C,H,W] -> fT[w'', r, c] with r over [h0-2, h0+NH+2), zero-padded."""
    ft = np.zeros((WP, NROWS, C), np.float32)
    r_lo, r_hi = h0 - 2, h0 + NH + 2
    s_lo, s_hi = max(r_lo, 0), min(r_hi, H)
    ft[PAD : PAD + W, s_lo - r_lo : s_hi - r_lo, :] = feat_n[:, s_lo:s_hi, :].transpose(
        2, 1, 0
    )
    return ft.reshape(WP, NROWS * C).astype(ml_dtypes.bfloat16)


def _prep_msk3(masks_n: np.ndarray) -> np.ndarray:
    """[25, 2H, 2W] -> msk3[w', h, i, t20]  [WP, H, KS, RUN]
    t20 = (w - (w'-4))*4 + b*2 + a; value = masks[5i + (4 - t20//4), 2h+a, 2w+b]
    """
    tt = np.arange(RUN)
    wpp = np.arange(WP)
    dw = tt // 4
    b = (tt % 4) // 2
    a = tt % 2
    j = 4 - dw
    wup = 2 * (wpp[:, None] - 4 + dw[None, :]) + b[None, :]
    wup_c = np.clip(wup, 0, 2 * W - 1)                     # [WP, RUN]
    i_ar = np.arange(KS)
    k_full = 5 * i_ar[:, None] + j[None, :]                # [KS, RUN]
    hh = np.arange(H)
    hup = 2 * hh[:, None] + a[None, :]                     # [H, RUN]
    out = masks_n[
        k_full[None, None, :, :],
        hup[None, :, None, :],
        wup_c[:, None, None, :],
    ]  # [WP, H, KS, RUN]
    return out.astype(np.float32)


def kernel(features: np.ndarray, masks: np.ndarray, _perf: dict | None = None):
    features = np.asarray(features, dtype=np.float32)
    masks = np.asarray(masks, dtype=np.float32)

    if "nc" not in _PROG_CACHE:
        _PROG_CACHE["nc"] = _build_program()
    nc = _PROG_CACHE["nc"]

    in_maps = []
    for core in range(8):
        n, half = divmod(core, 2)
        h0 = NH * half
        ft_sh = _prep_ft(features[n], h0)
        msk3 = _prep_msk3(masks[n])[:, h0 : h0 + NH]  # [WP, NH, KS, RUN]
        in_maps.append(
            {
                "ft": ft_sh,
                "msk3": np.ascontiguousarray(
                    msk3.reshape(WP, MSK_COLS)
                ).astype(ml_dtypes.bfloat16),
            }
        )

    trace = bool(_perf is not None and _perf.get("trace"))
    res = run_bass_kernel_spmd(
        nc, in_maps, core_ids=list(range(8)), trace=trace,
        **({} if not trace else {"trace_cores": [0]}),
    )
    if _perf is not None:
        _perf["exec_time_ns"] = res.exec_time_ns
        _perf["trace"] = res.instructions_and_trace

    out = np.empty((N, C, SCALE * H, SCALE * W), np.float32)
    for core in range(8):
        n, half = divmod(core, 2)
        out[n, :, 64 * half : 64 * half + 64, :] = np.asarray(
            res.results[core]["out"]
        ).astype(np.float32)
    return out
